# revision 25
# baseline (speedup 1.0000x reference)
"""Trainium2 Bass kernel for the Desimilar block (retrieval_knn).

Pipeline per batch image (B=4, C=64):
  conv_down(4x4/s4) + BN + ReLU -> x1 [64, 64, 64]
  windowed (7x7) most-dissimilar top-9 neighbor mean -> out_img
  conv3x3([x1; out_img]) + BN + ReLU -> feat
  SE attention (global mean -> 1x1 conv -> BN -> sigmoid) -> feat * att
  bilinear x4 upsample -> [64, 256, 256]

Sharding: 8 cores = 4 batches x 2 H-halves (with recomputed halos).
The only cross-core exchange is a [64]-float AllReduce between H-half pairs
for the SE global mean.

Key kernel tricks:
  * ranking by (sq_m - 2*dot) == ranking by similarity (monotone, per-row
    shift invariant) -> no sqrt/reciprocal needed.
  * distance rows via matmul with augmented K: lhsT=[-2*x1; 1], rhs=[x1; sq]
  * window extraction from the row-major distance block via gpsimd
    indirect_copy (per-partition index vectors = diagonal gather).
  * top-9 threshold via DVE max (top-8) + match_replace + reduce_max.
  * selection matrix built TRANSPOSED directly by a second matmul family
    (same SBUF operands, roles swapped) with -t' folded in as an extra
    contraction row -> no PE transposes, no partition broadcasts.
  * gather-sum of selected features as matmul over the 0/1 selection.
  * "- x1" term of out_img folded into the conv3x3 weights host-side.
  * bilinear x4 as one matmul family: 4 output-column phases, each
    K=(3 shifted copies x 34 rows), weights from the exact jax resize matrix.
"""

import numpy as np
from contextlib import ExitStack

import concourse.bass as bass
import concourse.bacc as bacc
import concourse.tile as tile
from concourse import mybir
from concourse.bass_utils import run_bass_kernel_spmd

F32 = mybir.dt.float32
U16 = mybir.dt.uint16
AF = mybir.ActivationFunctionType
ALU = mybir.AluOpType
AX = mybir.AxisListType

# ---------------------------------------------------------------- geometry
B, C, HIN = 4, 64, 256
H = W = 64          # downsampled
EXP, TOPK, DS, UP = 3, 9, 4, 4
WIN = 2 * EXP + 1   # 7
NCORES = 8

HALF = 32           # feat rows of the output region per core
X1R = 42            # x1 rows per core (halo 5 top+bottom)
GR0 = 3             # gather region = local rows [3, 39)
GROWS = 36
NBLK = 18           # 18 blocks of 128 positions
L1 = X1R * W        # 2688
L1P = L1 + 128      # 2816 padded (64 zero cols each side)
INROWS = 4 * X1R    # 168
FROWS = 34          # feat rows (= output halo 1)
CATS = GROWS        # cat h slots
CATW = 66
CATLEN = CATS * CATW + 2  # +2 slack for tap reads past the end
NEG = -3.0e38
PENV = 3.3e38


def _upmat():
    """U[hi, o] fp32: jax.image.resize bilinear 64->256 along one axis."""
    U = np.zeros((H, H * UP), np.float64)
    for o in range(H * UP):
        c = (o + 0.5) / UP - 0.5
        i0 = int(np.floor(c))
        w1 = c - i0
        U[min(max(i0, 0), H - 1), o] += 1.0 - w1
        U[min(max(i0 + 1, 0), H - 1), o] += w1
    return U.astype(np.float32)


# ---------------------------------------------------------------- program
def build_program(dbg=False):
    nc = bacc.Bacc("TRN2", target_bir_lowering=False, debug=False,
                   num_devices=NCORES)
    if dbg:
        d_x1r = nc.declare_dram_parameter("d_x1r", [66, L1P], F32, isOutput=True)
        d_cat = nc.declare_dram_parameter("d_cat", [128, CATLEN], F32,
                                          isOutput=True)
        d_feat = nc.declare_dram_parameter("d_feat", [64, FROWS * W], F32,
                                           isOutput=True)
        d_win = nc.declare_dram_parameter("d_win", [128, NBLK, 49], F32,
                                          isOutput=True)
        d_t9 = nc.declare_dram_parameter("d_t9", [128, NBLK], F32,
                                         isOutput=True)
        d_st = nc.declare_dram_parameter("d_st", [128, NBLK, 640], F32,
                                         isOutput=True)
        d_att = nc.declare_dram_parameter("d_att", [64, 1], F32, isOutput=True)
        d_rhs3 = nc.declare_dram_parameter("d_rhs3", [102, 64, 64], F32,
                                           isOutput=True)

    xs = nc.declare_dram_parameter("xs", [C, INROWS, HIN], F32, isOutput=False)
    wdp = nc.declare_dram_parameter("wdp", [128, 8, 64], F32, isOutput=False)
    w1p = nc.declare_dram_parameter("w1p", [128, 9, 64], F32, isOutput=False)
    waT = nc.declare_dram_parameter("waT", [64, 64], F32, isOutput=False)
    sdv = nc.declare_dram_parameter("sdv", [64, 1], F32, isOutput=False)
    bdv = nc.declare_dram_parameter("bdv", [64, 1], F32, isOutput=False)
    s1v = nc.declare_dram_parameter("s1v", [64, 1], F32, isOutput=False)
    b1v = nc.declare_dram_parameter("b1v", [64, 1], F32, isOutput=False)
    sav = nc.declare_dram_parameter("sav", [64, 1], F32, isOutput=False)
    bav = nc.declare_dram_parameter("bav", [64, 1], F32, isOutput=False)
    idxw = nc.declare_dram_parameter("idxw", [128, 49], U16, isOutput=False)
    admk = nc.declare_dram_parameter("admk", [128, NBLK, 49], F32, isOutput=False)
    penm = nc.declare_dram_parameter("penm", [128, NBLK], F32, isOutput=False)
    bandt = nc.declare_dram_parameter("bandt", [128, 5, 128], F32, isOutput=False)
    lhsTr = nc.declare_dram_parameter("lhsTr", [102, 4, 128], F32, isOutput=False)
    idn = nc.declare_dram_parameter("idn", [128, 128], F32, isOutput=False)
    onesv = nc.declare_dram_parameter("onesv", [1, L1P], F32, isOutput=False)

    outp = nc.declare_dram_parameter("outp", [4, C, 128, 64], F32, isOutput=True)

    with tile.TileContext(nc) as tc, ExitStack() as ctx:
        cst = ctx.enter_context(tc.tile_pool(name="cst", bufs=1))
        big = ctx.enter_context(tc.tile_pool(name="big", bufs=1))
        dpool = ctx.enter_context(tc.tile_pool(name="dram", bufs=1, space="DRAM"))

        # ---- consts to SBUF
        wdp_sb = cst.tile([128, 8, 64], F32)
        nc.sync.dma_start(wdp_sb[:, :, :], wdp[:, :, :])
        w1p_sb = cst.tile([128, 9, 64], F32)
        nc.sync.dma_start(w1p_sb[:, :, :], w1p[:, :, :])
        waT_sb = cst.tile([64, 64], F32)
        nc.sync.dma_start(waT_sb[:, :], waT[:, :])
        sd_sb = cst.tile([64, 1], F32); nc.sync.dma_start(sd_sb[:, :], sdv[:, :])
        bd_sb = cst.tile([64, 1], F32); nc.sync.dma_start(bd_sb[:, :], bdv[:, :])
        s1_sb = cst.tile([64, 1], F32); nc.sync.dma_start(s1_sb[:, :], s1v[:, :])
        b1_sb = cst.tile([64, 1], F32); nc.sync.dma_start(b1_sb[:, :], b1v[:, :])
        sa_sb = cst.tile([64, 1], F32); nc.sync.dma_start(sa_sb[:, :], sav[:, :])
        ba_sb = cst.tile([64, 1], F32); nc.sync.dma_start(ba_sb[:, :], bav[:, :])
        adm_sb = cst.tile([128, NBLK, 49], F32)
        nc.sync.dma_start(adm_sb[:, :, :], admk[:, :, :])
        pen_sb = cst.tile([128, NBLK], F32)
        nc.sync.dma_start(pen_sb[:, :], penm[:, :])
        band_sb = cst.tile([128, 5, 128], F32)
        nc.sync.dma_start(band_sb[:, :, :], bandt[:, :, :])
        lr_sb = cst.tile([102, 4, 128], F32)
        nc.sync.dma_start(lr_sb[:, :, :], lhsTr[:, :, :])
        idn_sb = cst.tile([128, 128], F32)
        nc.sync.dma_start(idn_sb[:, :], idn[:, :])
        ones_sb = cst.tile([64, 1], F32)
        nc.vector.memset(ones_sb[:, :], 1.0)

        # ---- persistent tensors
        x1r = big.tile([66, L1P], F32)     # [x1; sq; ones] padded 64 each side
        x1a = big.tile([66, L1], F32)      # [-2*x1; ones; -t'' row]
        x1T = big.tile([128, 22 * 64], F32)
        x1sq = big.tile([64, L1], F32)
        cat = big.tile([128, CATLEN], F32)
        feat = big.tile([64, FROWS * W], F32)
        psums = big.tile([64, 6], F32)     # 5 conv3x3 partial sums + reduced
        att_sb = big.tile([64, 1], F32)
        sum2_sb = big.tile([64, 1], F32)
        rhs3 = big.tile([102, 64, 64], F32)

        feat_dram = dpool.tile([C, FROWS, W], F32)

        # initial memsets (engine ops need 32-aligned partition bases; row 65
        # of x1r comes from a host-side ones vector via DMA)
        nc.vector.memset(x1r[0:65, 0:64], 0.0)
        nc.vector.memset(x1r[0:65, L1 + 64:L1P], 0.0)
        nc.sync.dma_start(x1r[65:66, 0:L1P], onesv[:, :])
        nc.vector.memset(x1a[64:65, 0:L1], 1.0)
        nc.gpsimd.memset(cat[0:128, 0:CATLEN], 0.0)

        # ============================================= Phase A: conv_down
        with tc.tile_pool(name="cdin", bufs=3) as cdin, \
             tc.tile_pool(name="cdps", bufs=2, space="PSUM") as cdps:
            for g in range(7):
                tl = []
                for khb in range(2):
                    t = cdin.tile([128, 6, 64, 4], F32, tag="cdin")
                    for khh in range(2):
                        src = bass.AP(xs, (24 * g + 2 * khb + khh) * HIN,
                                      [[INROWS * HIN, 64], [4 * HIN, 6],
                                       [1, 256]])
                        nc.sync.dma_start(t[64 * khh:64 * khh + 64, :, :, :],
                                          src)
                    tl.append(t)
                ps = cdps.tile([64, 6, 64], F32)
                first = True
                for khb in range(2):
                    for kw in range(4):
                        nc.tensor.matmul(
                            ps[:, :, :],
                            wdp_sb[:, 4 * khb + kw, :],
                            tl[khb][:, :, :, kw],
                            start=first, stop=(khb == 1 and kw == 3))
                        first = False
                # x1 = relu(psum * sd + bd)
                nc.scalar.activation(
                    x1r[0:64, 64 + 384 * g:64 + 384 * (g + 1)],
                    ps[:, :, :], AF.Relu, bias=bd_sb[:, :], scale=sd_sb[:, :])
                xsl = x1r[0:64, 64 + 384 * g:64 + 384 * (g + 1)]
                nc.vector.tensor_scalar_mul(
                    x1a[0:64, 384 * g:384 * (g + 1)], xsl, -2.0)
                nc.vector.tensor_mul(
                    x1sq[0:64, 384 * g:384 * (g + 1)], xsl, xsl)

        # copy x1 gather-region into cat rows 0:64 (w-interior)
        catv = cat[0:64, 0:CATS * CATW].rearrange("p (s w) -> p s w", w=CATW)
        x1v = x1r[0:64, 64 + GR0 * W:64 + (GR0 + CATS) * W].rearrange(
            "p (s w) -> p s w", w=W)
        nc.vector.tensor_copy(catv[:, :, 1:65], x1v[:, :, :])

        # ============================================= Phase B: sq row + x1T
        with tc.tile_pool(name="sqps", bufs=2, space="PSUM") as sqps, \
             tc.tile_pool(name="xtps", bufs=2, space="PSUM") as xtps:
            for ch in range(6):
                n0 = 512 * ch
                n1 = min(L1, n0 + 512)
                ps = sqps.tile([1, 512], F32, tag="sq")
                nc.tensor.matmul(ps[0:1, 0:n1 - n0], ones_sb[:, :],
                                 x1sq[:, n0:n1], start=True, stop=True)
                nc.scalar.activation(x1r[64:65, 64 + n0:64 + n1],
                                     ps[0:1, 0:n1 - n0], AF.Copy)
            for j in range(22):
                ps = xtps.tile([128, 64], F32, tag="xt")
                nc.tensor.transpose(ps[:, :], x1r[0:64, 128 * j:128 * (j + 1)],
                                    idn_sb[0:64, 0:64])
                nc.scalar.activation(x1T[:, 64 * j:64 * (j + 1)], ps[:, :],
                                     AF.Copy)

        # ============================================= Phase C: knn blocks
        with tc.tile_pool(name="psA", bufs=2, space="PSUM") as psA, \
             tc.tile_pool(name="psB", bufs=1, space="PSUM") as psB, \
             tc.tile_pool(name="psG", bufs=2, space="PSUM") as psG, \
             tc.tile_pool(name="d2p", bufs=2) as d2p, \
             tc.tile_pool(name="ddram", bufs=2, space="DRAM") as ddram, \
             tc.tile_pool(name="wins", bufs=2) as wins, \
             tc.tile_pool(name="stp", bufs=2) as stp:

            def emit_mm1(b):
                ps = psA.tile([128, 640], F32, tag="A")
                lh = x1a[0:65, 192 + 128 * b:320 + 128 * b]
                nc.tensor.matmul(ps[:, 0:512], lh,
                                 x1r[0:65, 128 * b:128 * b + 512],
                                 start=True, stop=True)
                nc.tensor.matmul(ps[:, 512:640], lh,
                                 x1r[0:65, 128 * b + 512:128 * b + 640],
                                 start=True, stop=True)
                return ps

            def emit_rest(b, psa):
                d2sb = d2p.tile([128, 640], F32, tag="d2")
                nc.scalar.activation(d2sb[:, :], psa[:, :], AF.Copy)
                d2d = ddram.tile([128, 640], F32, tag="d2d")
                nc.sync.dma_start(d2d[:, :], d2sb[:, :])
                win = wins.tile([128, 49], F32, tag="win")
                dd = d2d[:, :]
                diag = bass.AP(dd.tensor, dd.offset + 61,
                               [[641, 128], [64, 7], [1, 7]])
                nc.sync.dma_start(win[:, :], diag)
                if dbg:
                    nc.sync.dma_start(
                        bass.AP(d_win, b * 49, [[NBLK * 49, 128], [1, 49]]),
                        win[:, :])
                winm = wins.tile([128, 49], F32, tag="winm")
                nc.vector.tensor_add(winm[:, :], win[:, :], adm_sb[:, b, :])
                t8 = wins.tile([128, 8], F32, tag="t8")
                nc.vector.max(t8[:, :], winm[:, :])
                winr = wins.tile([128, 49], F32, tag="winr")
                nc.vector.match_replace(winr[:, :], t8[:, :], winm[:, :], NEG)
                t89 = wins.tile([128, 8], F32, tag="t89")
                nc.vector.max(t89[:, :], winr[:, :])
                t9 = wins.tile([128, 1], F32, tag="t9")
                nc.vector.tensor_add(t9[:, :], t89[:, 0:1], t89[:, 1:2])
                if dbg:
                    nc.sync.dma_start(
                        bass.AP(d_t9, b, [[NBLK, 128], [1, 1]]), t9[:, :])
                ngt = wins.tile([128, 1], F32, tag="ngt")
                nc.vector.scalar_tensor_tensor(
                    ngt[:, :], t9[:, :], -0.5, pen_sb[:, b:b + 1],
                    op0=ALU.mult, op1=ALU.subtract)
                pst = psG.tile([128, 128], F32, tag="G")
                nc.tensor.transpose(pst[0:1, 0:128], ngt[:, :],
                                    idn_sb[0:128, 0:128])
                trow = wins.tile([1, 128], F32, tag="trow")
                nc.scalar.activation(trow[:, :], pst[0:1, 0:128], AF.Copy)
                nc.sync.dma_start(x1a[65:66, 192 + 128 * b:320 + 128 * b],
                                  trow[:, :])
                # D2^T chunks + selection + gather
                psb = psB.tile([128, 640], F32, tag="B")
                st = stp.tile([128, 640], F32, tag="st")
                for k in range(5):
                    nc.tensor.matmul(
                        psb[:, 128 * k:128 * (k + 1)],
                        x1r[0:66, 128 * (b + k):128 * (b + k) + 128],
                        x1a[0:66, 192 + 128 * b:320 + 128 * b],
                        start=True, stop=True)
                    nc.vector.tensor_tensor(
                        st[:, 128 * k:128 * (k + 1)],
                        psb[:, 128 * k:128 * (k + 1)],
                        band_sb[:, k, :], ALU.is_ge)
                if dbg:
                    nc.sync.dma_start(
                        bass.AP(d_st, b * 640, [[NBLK * 640, 128], [1, 640]]),
                        st[:, :])
                psg = psG.tile([128, 128], F32, tag="G")
                for k in range(5):
                    nc.tensor.matmul(
                        psg[64:128, :],
                        x1T[:, 64 * (b + k):64 * (b + k) + 64],
                        st[:, 128 * k:128 * (k + 1)],
                        start=(k == 0), stop=(k == 4),
                        tile_position=(0, 64))
                catd = cat[64:128, 0:CATS * CATW].rearrange(
                    "p (s w) -> p s w", w=CATW)
                nc.scalar.activation(
                    catd[:, 2 * b:2 * b + 2, 1:65],
                    psg[64:128, :].rearrange("p (s w) -> p s w", w=W),
                    AF.Copy, scale=1.0 / TOPK)

            prev = emit_mm1(0)
            for b in range(NBLK):
                nxt = emit_mm1(b + 1) if b + 1 < NBLK else None
                emit_rest(b, prev)
                prev = nxt

        if dbg:
            nc.sync.dma_start(bass.AP(d_x1r, 0, [[L1P, 66], [1, L1P]]),
                              x1r[:, :])
            nc.sync.dma_start(bass.AP(d_cat, 0, [[CATLEN, 128], [1, CATLEN]]),
                              cat[:, :])

        # ============================================= Phase D: conv3x3
        chunk_slots = [7, 7, 7, 7, 6]
        with tc.tile_pool(name="c3ps", bufs=2, space="PSUM") as c3ps:
            s0 = 0
            for ci, ns in enumerate(chunk_slots):
                ps = c3ps.tile([64, 7, CATW], F32, tag="c3")
                first = True
                for kh in range(3):
                    for kw in range(3):
                        off = kh * CATW + kw + s0 * CATW
                        nc.tensor.matmul(
                            ps[:, 0:ns, :],
                            w1p_sb[:, kh * 3 + kw, :],
                            cat[:, off:off + ns * CATW],
                            start=first, stop=(kh == 2 and kw == 2))
                        first = False
                nc.scalar.activation(
                    feat[:, s0 * W:(s0 + ns) * W],
                    ps[:, 0:ns, 0:64], AF.Relu,
                    bias=b1_sb[:, :], scale=s1_sb[:, :])
                s0 += ns

        # ============================================= Phase E: SE attention
        # mean over exactly this core's 32 image rows = feat rows [1, 33)
        nc.vector.reduce_sum(psums[:, 5:6], feat[:, W:33 * W], axis=AX.X)
        cc_in = dpool.tile([64, 1], F32)
        cc_out = dpool.tile([64, 1], F32)
        nc.sync.dma_start(cc_in[:, :], psums[:, 5:6])
        nc.gpsimd.collective_compute(
            "AllReduce", ALU.add,
            replica_groups=[[0, 1], [2, 3], [4, 5], [6, 7]],
            ins=[cc_in.opt()],
            outs=[cc_out.opt()],
        )
        nc.sync.dma_start(sum2_sb[:, :], cc_out[:, :])

        with tc.tile_pool(name="atps", bufs=1, space="PSUM") as atps:
            ps = atps.tile([64, 1], F32)
            nc.tensor.matmul(ps[:, :], waT_sb[:, :], sum2_sb[:, :],
                             start=True, stop=True)
            nc.scalar.activation(att_sb[:, :], ps[:, :], AF.Sigmoid,
                                 bias=ba_sb[:, :], scale=sa_sb[:, :])
        if dbg:
            nc.sync.dma_start(
                bass.AP(d_feat, 0, [[FROWS * W, 64], [1, FROWS * W]]),
                feat[:, :])
            nc.sync.dma_start(bass.AP(d_att, 0, [[1, 64], [1, 1]]),
                              att_sb[:, :])
        nc.vector.tensor_scalar_mul(feat[:, :], feat[:, :], att_sb[:, :])

        # ============================================= Phase F: upsample
        nc.sync.dma_start(feat_dram[:, :, :],
                          feat[:, :].rearrange("p (h w) -> p h w", w=W))
        # fdT: [h, c, w] view of feat_dram for the h-partition reloads
        fdT = feat_dram[:, :, :].transpose([1, 0, 2])
        for s in range(3):
            d = s - 1
            if d == 0:
                nc.sync.dma_start(rhs3[34 * s:34 * s + 34, :, :], fdT[:, :, :])
            elif d == -1:
                nc.sync.dma_start(rhs3[34 * s:34 * s + 34, :, 1:64],
                                  fdT[:, :, 0:63])
                nc.sync.dma_start(rhs3[34 * s:34 * s + 34, :, 0:1],
                                  fdT[:, :, 0:1])
            else:
                nc.sync.dma_start(rhs3[34 * s:34 * s + 34, :, 0:63],
                                  fdT[:, :, 1:64])
                nc.sync.dma_start(rhs3[34 * s:34 * s + 34, :, 63:64],
                                  fdT[:, :, 63:64])
        if dbg:
            nc.sync.dma_start(
                bass.AP(d_rhs3, 0, [[4096, 102], [1, 4096]]), rhs3[:, :, :])
        with tc.tile_pool(name="upps", bufs=4, space="PSUM") as upps, \
             tc.tile_pool(name="upsb", bufs=4) as upsb:
            for r in range(4):
                for ch in range(8):
                    ps = upps.tile([128, 8, 64], F32, tag="up")
                    nc.tensor.matmul(ps[:, :, :], lr_sb[:, r, :],
                                     rhs3[:, 8 * ch:8 * (ch + 1), :],
                                     start=True, stop=True)
                    ob = upsb.tile([128, 8, 64], F32, tag="ob")
                    if ch % 2 == 0:
                        nc.scalar.activation(ob[:, :, :], ps[:, :, :], AF.Copy)
                    else:
                        nc.vector.tensor_copy(ob[:, :, :], ps[:, :, :])
                    dst = bass.AP(outp,
                                  r * (C * 128 * 64) + 8 * ch * (128 * 64),
                                  [[64, 128], [128 * 64, 8], [1, 64]])
                    nc.sync.dma_start(dst, ob[:, :, :])

    nc.compile()
    return nc


# ---------------------------------------------------------------- host prep
def _host_consts():
    """Core-independent constant tensors."""
    # window extraction indices
    idxw = np.zeros((128, 49), np.uint16)
    for l in range(128):
        for k in range(7):
            for d in range(7):
                idxw[l, k * 7 + d] = l + 64 * k + d + 61
    # transposed band threshold (0 in valid window, +inf outside)
    bandt = np.full((128, 5, 128), np.inf, np.float32)
    for k in range(5):
        for ml in range(128):
            for ll in range(128):
                diff = 128 * k + ml - 256 - ll
                ko = int(np.round(diff / 64.0))
                dd = diff - 64 * ko
                if abs(ko) <= 3 and abs(dd) <= 3:
                    lcol = ll % 64
                    if 0 <= lcol + dd < 64:
                        bandt[ml, k, ll] = 0.0
    idn = np.eye(128, dtype=np.float32)
    return idxw, bandt, idn


def _core_masks(half):
    """addmask [128, 18, 49] and pen [128, 18] for one H-half."""
    r0 = HALF * half
    admk = np.full((128, NBLK, 49), NEG, np.float32)
    pen = np.zeros((128, NBLK), np.float32)
    for b in range(NBLK):
        for l in range(128):
            grow = r0 - 2 + 2 * b + l // 64
            col = l % 64
            if grow < 0 or grow >= H:
                pen[l, b] = PENV
                continue
            for k in range(7):
                for d in range(7):
                    nr = grow + k - 3
                    ncol = col + d - 3
                    if 0 <= nr < H and 0 <= ncol < W:
                        admk[l, b, k * 7 + d] = 0.0
    return admk, pen


def _core_upweights(half):
    """lhsT_r [102, 4, 128] from the exact bilinear matrix."""
    U = _upmat()                      # [64, 256]
    r0 = HALF * half
    UH = np.zeros((FROWS, 128), np.float32)
    for gi, g in enumerate(range(r0 - 1, r0 + 33)):
        if 0 <= g < H:
            UH[gi] = U[g, 128 * half:128 * half + 128]
    # W-phase constants from an interior column
    q0 = 32
    wr = np.zeros((4, 3), np.float32)     # [r, s] with s=dq+1
    for r in range(4):
        for s in range(3):
            wr[r, s] = U[q0 + s - 1, 4 * q0 + r]
    lhsTr = np.zeros((102, 4, 128), np.float32)
    for r in range(4):
        for s in range(3):
            lhsTr[34 * s:34 * s + 34, r, :] = UH * wr[r, s]
    return lhsTr


_PROG_CACHE = {}


def kernel(x, wd, bd, bnd_g, bnd_b, bnd_m, bnd_v,
           w1, b1, bn1_g, bn1_b, bn1_m, bn1_v,
           wa, bna_g, bna_b, bna_m, bna_v):
    x = np.asarray(x, np.float32)
    eps = 1e-5

    if "nc" not in _PROG_CACHE:
        _PROG_CACHE["nc"] = build_program()
    nc = _PROG_CACHE["nc"]

    # ---- folded BN scales
    sd = np.asarray(bnd_g) / np.sqrt(np.asarray(bnd_v) + eps)
    bd_f = np.asarray(bnd_b) + (np.asarray(bd) - np.asarray(bnd_m)) * sd
    s1 = np.asarray(bn1_g) / np.sqrt(np.asarray(bn1_v) + eps)
    b1_f = np.asarray(bn1_b) + (np.asarray(b1) - np.asarray(bn1_m)) * s1
    sa = np.asarray(bna_g) / np.sqrt(np.asarray(bna_v) + eps)
    ba_f = np.asarray(bna_b) - np.asarray(bna_m) * sa
    sa = sa / float(H * W)

    # ---- weights repack
    wd_np = np.asarray(wd, np.float32)      # [co, ci, kh, kw]
    wdp = np.zeros((128, 8, 64), np.float32)
    for khb in range(2):
        for kw in range(4):
            for khh in range(2):
                # partition = khh*64 + ci
                wdp[64 * khh:64 * khh + 64, 4 * khb + kw, :] = \
                    wd_np[:, :, 2 * khb + khh, kw].T
    w1_np = np.asarray(w1, np.float32)      # [co, 2c, kh, kw]
    # fold out_img = G/9 - x1: w1'[:, :64] -= w1[:, 64:]
    w1_m = w1_np.copy()
    w1_m[:, 0:64] = w1_np[:, 0:64] - w1_np[:, 64:128]
    w1p = np.zeros((128, 9, 64), np.float32)
    for kh in range(3):
        for kw in range(3):
            w1p[:, kh * 3 + kw, :] = w1_m[:, :, kh, kw].T
    waT_np = np.asarray(wa, np.float32)[:, :, 0, 0].T.copy()

    idxw, bandt, idn = _host_consts()

    in_maps = []
    for core in range(NCORES):
        b = core // 2
        half = core % 2
        g0 = HALF * half - 5
        xi0 = 4 * g0
        xs = np.zeros((C, INROWS, HIN), np.float32)
        lo = max(0, xi0)
        hi = min(HIN, xi0 + INROWS)
        xs[:, lo - xi0:hi - xi0, :] = x[b, :, lo:hi, :]
        admk, pen = _core_masks(half)
        in_maps.append(dict(
            xs=xs,
            wdp=wdp, w1p=w1p, waT=waT_np,
            sdv=sd.reshape(64, 1).astype(np.float32),
            bdv=bd_f.reshape(64, 1).astype(np.float32),
            s1v=s1.reshape(64, 1).astype(np.float32),
            b1v=b1_f.reshape(64, 1).astype(np.float32),
            sav=sa.reshape(64, 1).astype(np.float32),
            bav=ba_f.reshape(64, 1).astype(np.float32),
            idxw=idxw, admk=admk, penm=pen, bandt=bandt,
            lhsTr=_core_upweights(half), idn=idn,
            onesv=np.ones((1, L1P), np.float32),
        ))

    _PROG_CACHE["in_maps"] = in_maps
    res = run_bass_kernel_spmd(nc, in_maps, list(range(NCORES)))
    _PROG_CACHE["last_result"] = res
    results = res.results

    out = np.empty((B, C, HIN, HIN), np.float32)
    for core in range(NCORES):
        b = core // 2
        half = core % 2
        ph = results[core]["outp"]          # [4, C, 128, 64]
        blk = ph.transpose(1, 2, 3, 0).reshape(C, 128, 256)
        out[b, :, 128 * half:128 * half + 128, :] = blk
    return out


if __name__ == "__main__":
    import reference
    inputs = {k: np.asarray(v) for k, v in reference.setup_inputs().items()}
    got = kernel(**inputs)
    exp = np.asarray(reference.reference(**inputs))
    err = np.linalg.norm(got - exp) / np.linalg.norm(exp)
    print("Relative error:", err)
